# revision 19
# baseline (speedup 1.0000x reference)
"""BitLinear TRN2 kernel v10: y = x @ W(pweight,nweight)^T + bias.

Sharding: 2 token-shards x 4 out-feature shards (column-parallel linear,
no collectives). Each core: 8192 tokens x 512 out features.

Structure:
- PE warm-up dummies from t=0 hold the HAM clock gate at 2.4 GHz until the
  first real matmul (~24us).
- Weight bit-planes split by significance: plane 0 ships bf16, planes
  1-3 ship fp8-e4m3 (sigmoid-input quantization of the low planes moves
  the result by <5e-3 rel; planes are weighted 8:4:2:1). Cuts the
  weight stream 16MB -> 10MB, which un-starves the serial ACT sigmoid
  chain (weights feed it at ~200GB/s across three DMA rings).
- Ring split in sigmoid order (tiles t0..t7 = (ot,h)): sync carries t0,t1
  then all x; scalar t2,t4,t6; gpsimd t3,t5,t7 then y.
- x host-prepacked slab-contiguous (16KB descriptor lines), slabs paced
  behind the sigmoid chain via manual deps so early HBM bandwidth goes to
  the weights the chain is waiting on.
- o-tile-major periods [2,3,3] over 1024-token slabs: phase0 runs on just
  8MB of x (no front burst), and wT[ot_k] deadlines stretch to ~14+21k us.
- Combine fused as 8 signed planes via scalar_tensor_tensor mult+add on
  DVE; PSUM 4-parity rotation; drains are DVE tensor_scalar(PSUM +
  per-partition bias) -> bf16 yT.
Output yT [512, 8192] bf16, upcast + transposed on host.
"""

import numpy as np

import concourse.bass as bass
import concourse.mybir as mybir
import concourse.tile as tile
from concourse.tile import add_dep_helper
from concourse import bacc
from concourse.bass_utils import run_bass_kernel_spmd

N_CORES = 8
T, I, O, NB = 16384, 2048, 2048, 4
R, C = 2, 4  # token shards x out-feature shards
TQ = T // R  # 8192 tokens per core
OC = O // C  # 512 out features per core
P = 128
N_IT = I // P  # 16 i-tiles
N_OT = OC // P  # 4 o-tiles per core
N_H = 2  # i-halves per prep tile
HIT = N_IT // N_H  # 8 i-tiles per half
NBF = 1  # bf16 bit-planes (plane 0)
NF8 = NB - NBF  # fp8 bit-planes (planes 1..3)
TSLAB = 1024  # tokens per slab
N_SLAB = TQ // TSLAB  # 8 slabs
TCH = 512  # moving free size per matmul
N_TC = TSLAB // TCH  # 2 t-chunks = 2 PSUM banks per (slab, ot) group
PERIODS = [2, 3, 3]  # slabs per period (o-tile-major within a period)
N_WARM = 70  # dummy PE warm-up ldweights+matmul pairs
N_WBUF = 4  # weight tile pool depth (t0,t1 sync; t2,t3 pre-issued scalar)
# x slab k: issue after sigmoid op X_PACE[k] (None = immediately)
X_PACE = [None, None, 9, 11, 13, 15, 15, 15]
N_YSYNC = 4  # trailing y DMAs moved to the (idle by then) sync ring
DT = mybir.dt.bfloat16
F8 = mybir.dt.float8e4
F32 = mybir.dt.float32

_BUILT = None


def _build_bass():
    nc = bacc.Bacc("TRN2", debug=False, num_devices=N_CORES)

    # x prepacked: [N_SLAB, P, N_IT*TSLAB]: half-slab DMA = 16KB lines
    xp_d = nc.dram_tensor("xp", [N_SLAB, P, N_IT * TSLAB], DT, kind="ExternalInput").ap()
    # weights: plane 0 bf16, planes 1-3 fp8, p/n packed per (ot,h)
    wbf_d = nc.dram_tensor(
        "wbf", [N_OT, N_H, P, 2, HIT, P], DT, kind="ExternalInput"
    ).ap()
    wf8_d = nc.dram_tensor(
        "wf8", [N_OT, N_H, P, 2, HIT, NF8, P], F8, kind="ExternalInput"
    ).ap()
    # signed combine coefficients, STT plane order:
    # [+c0, -c0, +c1, +c2, +c3, -c1, -c2, -c3]
    cv_d = nc.dram_tensor("cvec", [P, 2 * NB], F32, kind="ExternalInput").ap()
    bias_d = nc.dram_tensor("bias", [P, N_OT], F32, kind="ExternalInput").ap()
    y_d = nc.dram_tensor("y", [OC, TQ], DT, kind="ExternalOutput").ap()

    with tile.TileContext(nc) as tc:
        with (
            tc.tile_pool(name="const", bufs=1) as const_pool,
            tc.tile_pool(name="xs", bufs=6) as xs_pool,
            tc.tile_pool(name="wbf", bufs=N_WBUF) as wbf_pool,
            tc.tile_pool(name="wf8", bufs=N_WBUF) as wf8_pool,
            tc.tile_pool(name="sbf", bufs=2) as sbf_pool,
            tc.tile_pool(name="sf8", bufs=2) as sf8_pool,
            tc.tile_pool(name="acc", bufs=1) as acc_pool,
            tc.tile_pool(name="wT", bufs=1) as wt_pool,
            tc.tile_pool(name="yo", bufs=3) as yo_pool,
            tc.tile_pool(name="mm_ps", bufs=1, space="PSUM") as mm_ps,
        ):
            cv_sb = const_pool.tile([P, 2 * NB], F32)
            nc.gpsimd.dma_start(cv_sb[:], cv_d[:])
            bias_sb = const_pool.tile([P, N_OT], F32)
            nc.gpsimd.dma_start(bias_sb[:], bias_d[:])

            # ---------- PE warm-up (hold HAM at 8/8 while DMA+prep run) ----
            wdum = const_pool.tile([P, P], DT, name="wdum")
            xdum = const_pool.tile([P, TCH], DT, name="xdum")
            nc.vector.memset(wdum[:], 0.0)
            nc.vector.memset(xdum[:], 0.0)
            # parity-3 banks are first used by a real group at ~45us
            warm_ps = mm_ps.tile([P, TCH], F32, tag="ps3c0", name="ps3c0")
            for _ in range(N_WARM):
                nc.tensor.ldweights(wdum[:])
                mm = nc.tensor.matmul(
                    warm_ps[:], wdum[:], xdum[:], start=True, stop=True
                )
                mm.ldweights = False

            # ---------- weight prep ----------------------------------------
            wTs = [
                [
                    wt_pool.tile([P, HIT, P], DT, tag=f"wT{ot}{h}", name=f"wT{ot}{h}")
                    for h in range(N_H)
                ]
                for ot in range(N_OT)
            ]
            # weight DMAs: t0,t1 on sync (fast ring); t2..t7 on scalar, with
            # the first two pre-issued while the ACT engine is still idle
            # (descriptor generation competes with sigmoid ops otherwise) and
            # the rest emitted right after the sigmoid that frees their
            # pool buffer (N_WBUF rotation).
            def issue_w(t):
                ot, h = divmod(t, N_H)
                ring = nc.sync if t < 2 else nc.scalar
                wbf = wbf_pool.tile([P, 2, HIT, P], DT, tag="wbf")
                ring.dma_start(wbf[:], wbf_d[ot, h])
                wf8 = wf8_pool.tile([P, 2, HIT, NF8, P], F8, tag="wf8")
                ring.dma_start(wf8[:], wf8_d[ot, h])
                return wbf, wf8

            wtiles = [issue_w(t) for t in range(N_WBUF)]
            sigmas = []
            for t in range(N_OT * N_H):
                ot, h = divmod(t, N_H)
                wbf, wf8 = wtiles[t]
                sbf = sbf_pool.tile([P, 2, HIT, P], DT, tag="sbf")
                sigmas.append(
                    nc.scalar.activation(
                        sbf[:], wbf[:], mybir.ActivationFunctionType.Sigmoid
                    )
                )
                sf8 = sf8_pool.tile([P, 2, HIT, NF8, P], DT, tag="sf8")
                sigmas.append(
                    nc.scalar.activation(
                        sf8[:], wf8[:], mybir.ActivationFunctionType.Sigmoid
                    )
                )
                if t + N_WBUF < N_OT * N_H:
                    wtiles.append(issue_w(t + N_WBUF))
                acc = acc_pool.tile([P, HIT, P], F32, tag="acc")
                # STT plane order: bf p(+c0), bf n(-c0), f8 p(+c1..3),
                # f8 n(-c1..3) — matches host cvec layout
                planes = [sbf[:, 0], sbf[:, 1]]
                planes += [sf8[:, 0, :, j, :] for j in range(NF8)]
                planes += [sf8[:, 1, :, j, :] for j in range(NF8)]
                for k, plane in enumerate(planes):
                    ck = cv_sb[:, k : k + 1]
                    if k == 0:
                        nc.vector.tensor_scalar(
                            acc[:], plane, ck, None, mybir.AluOpType.mult
                        )
                    else:
                        dst = wTs[ot][h][:] if k == 2 * NB - 1 else acc[:]
                        nc.vector.scalar_tensor_tensor(
                            dst,
                            plane,
                            ck,
                            acc[:],
                            mybir.AluOpType.mult,
                            mybir.AluOpType.add,
                        )

            # ---------- x DMAs (sync ring, half-slab tiles, paced) ---------
            xtiles = []
            for sl in range(N_SLAB):
                halves = []
                half = HIT * TSLAB
                for h in range(N_H):
                    xt_sb = xs_pool.tile(
                        [P, HIT, TSLAB], DT, tag="xslab", name=f"x{sl}h{h}"
                    )
                    dma = nc.sync.dma_start(
                        xt_sb[:],
                        xp_d[sl, :, h * half : (h + 1) * half].rearrange(
                            "p (it t) -> p it t", t=TSLAB
                        ),
                    )
                    if X_PACE[sl] is not None:
                        add_dep_helper(
                            dma.ins,
                            sigmas[X_PACE[sl]].ins,
                            reason="pace x behind sigmoid chain",
                        )
                    halves.append(xt_sb)
                xtiles.append(halves)

            # ---------- main: o-tile-major within slab periods -------------
            g = 0  # (slab, ot) group index -> PSUM parity g % 4
            s0 = 0
            for plen in PERIODS:
                slabs = range(s0, s0 + plen)
                s0 += plen
                for ot in range(N_OT):
                    for sl in slabs:
                        par = g % 4
                        g += 1
                        banks = [
                            mm_ps.tile(
                                [P, TCH], F32, tag=f"ps{par}c{c}", name=f"ps{par}c{c}"
                            )
                            for c in range(N_TC)
                        ]
                        for h in range(N_H):
                            for itl in range(HIT):
                                it = h * HIT + itl
                                lw = wTs[ot][h][:, itl, :]
                                nc.tensor.ldweights(lw)
                                for c in range(N_TC):
                                    mm = nc.tensor.matmul(
                                        banks[c][:],
                                        lw,
                                        xtiles[sl][h][:, itl, c * TCH : (c + 1) * TCH],
                                        start=(it == 0),
                                        stop=(it == N_IT - 1),
                                    )
                                    mm.ldweights = False
                        yt = yo_pool.tile([P, TSLAB], DT, tag="yt")
                        bb = bias_sb[:, ot : ot + 1]
                        for c in range(N_TC):
                            nc.vector.tensor_scalar(
                                yt[:, c * TCH : (c + 1) * TCH],
                                banks[c][:],
                                bb,
                                None,
                                mybir.AluOpType.add,
                            )
                        yring = (
                            nc.sync
                            if g > N_SLAB * N_OT - N_YSYNC
                            else nc.gpsimd
                        )
                        yring.dma_start(
                            y_d[ot * P : (ot + 1) * P, sl * TSLAB : (sl + 1) * TSLAB],
                            yt[:],
                        )

    nc.compile()
    return nc


def get_built():
    global _BUILT
    if _BUILT is None:
        _BUILT = _build_bass()
    return _BUILT


def make_in_maps(
    input, pweight, nweight, exps, bexps, mask_weight, scale, pbias, nbias, biasscale
):
    import ml_dtypes

    input = np.asarray(input, dtype=np.float32)
    pweight = np.asarray(pweight, dtype=np.float32)
    nweight = np.asarray(nweight, dtype=np.float32)
    exps = np.asarray(exps, dtype=np.float32)
    bexps = np.asarray(bexps, dtype=np.float32)
    mask_weight = np.asarray(mask_weight, dtype=np.float32)
    scale = np.asarray(scale, dtype=np.float32)
    pbias = np.asarray(pbias, dtype=np.float32)
    nbias = np.asarray(nbias, dtype=np.float32)
    biasscale = np.asarray(biasscale, dtype=np.float32)

    mask = 1.0 / (1.0 + np.exp(-mask_weight))
    c4 = (exps * mask * scale[0]).astype(np.float32)
    # STT plane order: [+c0, -c0, +c1, +c2, +c3, -c1, -c2, -c3]
    c8 = np.concatenate([[c4[0], -c4[0]], c4[1:], -c4[1:]])
    cvec = np.ascontiguousarray(np.broadcast_to(c8, (P, 2 * NB)).astype(np.float32))

    bias_raw = (pbias - nbias) @ bexps  # [O]
    step = float(2**NB - 1)
    b = np.clip(bias_raw, -1.0, 1.0)
    bias = (np.round(np.abs(b) * step) / step * np.sign(b)) * biasscale[0]

    def wlayout(w, planes):
        # [OC, I, NB] -> [N_OT, N_H, P(part=i), HIT, nplanes, P(o)]
        a = w.reshape(N_OT, P, N_H, HIT, P, NB)[..., planes]
        return a.transpose(0, 2, 4, 3, 5, 1)  # [ot, h, p, hit, n, o]

    x = input.reshape(T, I)
    xps = []
    for tr in range(R):
        xt = x[tr * TQ : (tr + 1) * TQ].T.astype(ml_dtypes.bfloat16)  # [I, TQ]
        a = xt.reshape(N_IT, P, N_SLAB, TSLAB).transpose(2, 1, 0, 3)
        xps.append(np.ascontiguousarray(a.reshape(N_SLAB, P, N_IT * TSLAB)))

    in_maps = []
    for core in range(N_CORES):
        tr, oc = divmod(core, C)
        osl = slice(oc * OC, (oc + 1) * OC)
        pw_c, nw_c = pweight[osl], nweight[osl]
        # bf16 plane 0: [ot, h, p, hit, 1, o] -> [ot, h, p, 2(pn), hit, o]
        pbf = wlayout(pw_c.astype(ml_dtypes.bfloat16), [0])[:, :, :, :, 0, :]
        nbf = wlayout(nw_c.astype(ml_dtypes.bfloat16), [0])[:, :, :, :, 0, :]
        wbf = np.ascontiguousarray(np.stack([pbf, nbf], axis=3))
        # fp8 planes 1..3: [ot, h, p, 2(pn), hit, nf8, o]
        pf8 = wlayout(pw_c.astype(ml_dtypes.float8_e4m3), [1, 2, 3])
        nf8 = wlayout(nw_c.astype(ml_dtypes.float8_e4m3), [1, 2, 3])
        wf8 = np.ascontiguousarray(np.stack([pf8, nf8], axis=3))
        in_maps.append(
            {
                "xp": xps[tr],
                "wbf": wbf,
                "wf8": wf8,
                "cvec": cvec,
                "bias": np.ascontiguousarray(
                    bias[osl].reshape(N_OT, P).T.astype(np.float32)
                ),
            }
        )
    return in_maps


def gather_output(results):
    y = np.empty((T, O), dtype=np.float32)
    for core, r in enumerate(results):
        tr, oc = divmod(core, C)
        y[tr * TQ : (tr + 1) * TQ, oc * OC : (oc + 1) * OC] = (
            r["y"].astype(np.float32).T
        )
    return y.reshape(8, T // 8, O)


def kernel(**inputs) -> np.ndarray:
    in_maps = make_in_maps(**inputs)
    nc = get_built()
    res = run_bass_kernel_spmd(nc, in_maps, core_ids=list(range(N_CORES)))
    return gather_output(res.results)
